# revision 9
# baseline (speedup 1.0000x reference)
"""Trainium2 Bass kernel for adaptive_high_order_residual_v2 (ORDER=2 masked
sign-binarization, per-row stats).

Full-input contract: kernel(x, mask) takes the complete (4096, 11008) arrays,
shards rows across 8 NeuronCores (512 rows each; per-row reductions make this
embarrassingly parallel), runs one SPMD Bass program, and concatenates the
per-core outputs.

Math per row (exact restructuring of the reference, ORDER = 2):
  T     = x*m                                   (masked input; in-place on x)
  cnt   = sum(m); r1 = sum(T); r2 = sum(T^2)
  mean1 = r1/cnt ; var1 = r2/cnt - mean1^2 ; s1 = sqrt(var1 * 2/pi)
  ab    = |T - mean1|  (+ accum; invalid entries contribute |mean1| each,
                        corrected by scalar algebra)
  b1    = sign(T - mean1)      (unmasked: invalid entries give sign(-mean1))
  q     = (ab - s1) * b1       (+ accum sum q; invalid garbage is the
                                per-row constant s1*sign(mean1)-mean1,
                                corrected by scalar algebra)
  sum q^2 = (r2 - mean1*r1) + cnt*s1^2 - 2*s1*sum|T - mean1|_masked
            (no elementwise pass needed)
  mean2, s2 from the corrected sums; K = mean1 + mean2
  out   = (K + s1*b1 + s2*b2) * m,  b2 = sign(q - mean2)

Engine split per pass over the data (per-core totals at 128-partition rates,
ACT 36.7us/pass, DVE f32 45.9 / f16-TS 11.5 / f16-TT 22.9, Pool STT 61.2):
  ACT : square(T)+r2, abs+accum, sign1, sign2                  (~147us)
  DVE : T=x*m+r1 (STT), q (STT), u=s2*b2+K (TS 4x),
        bs1=s1*b1 (TS 4x), w=u+bs1 (TT 2x)                      (~138us)
  Pool: mask cast + cnt accum (TS), out = w*m -> f32 (STT)      (~122us)
  DMA : 50.7 MB @ 360 GB/s                                      (~141us)
"""

import sys

import numpy as np

sys.path.insert(0, "/opt/trn_rl_repo")

R = 512          # rows per core
N = 11008        # columns
P = 128          # SBUF partitions per row-block
NBLK = R // P    # 4 blocks per core
CW = 2752        # column chunk width
NCH = N // CW    # 4 chunks per block
NCORES = 8
C2 = 0.6366197723675814  # 2/pi
EPS = 1e-30

# Per-chunk engine assignment knobs ("act"/"dve"/"pool") for the flexible
# passes; index is chunk id within a block. Pool (gpsimd) only supports
# tensor_tensor and no-accum tensor_scalar, so CNT/SQ cannot go there.
CNT_ENGINE = ["act"] * NCH                     # mask cast + cnt accum
SQ_ENGINE = ["act", "act", "act", "act"]       # sum(T^2) accum
U_ENGINE = ["dve"] * NCH                      # u = s2*b2 + K (TS)
OUT_ENGINE = ["dve"] * NCH                    # out = w * m  (TT, f32 out)

_CACHE = {}


def _build_program():
    import concourse.bacc as bacc
    import concourse.mybir as mybir
    from concourse.tile import TileContext

    F32 = mybir.dt.float32
    F16 = mybir.dt.float16
    U8 = mybir.dt.uint8
    Alu = mybir.AluOpType
    Act = mybir.ActivationFunctionType

    nc = bacc.Bacc()
    x = nc.dram_tensor("x", [R, N], F32, kind="ExternalInput")
    mk = nc.dram_tensor("mask", [R, N], U8, kind="ExternalInput")
    out = nc.dram_tensor("out", [R, N], F32, kind="ExternalOutput")

    with TileContext(nc) as tc:
        with (
            tc.tile_pool(name="xq", bufs=2 * NCH) as xq_pool,   # x -> T -> q -> out
            tc.tile_pool(name="m8", bufs=2 * NCH) as m8_pool,   # u8 mask
            tc.tile_pool(name="b1", bufs=2 * NCH) as b1_pool,   # sign1 -> s1*b1 (f16)
            tc.tile_pool(name="ab", bufs=2) as ab_pool,         # |T - mean1| (f32)
            tc.tile_pool(name="b2", bufs=2) as b2_pool,         # sign2 -> u -> w (f16)
            tc.tile_pool(name="t16", bufs=2) as t16_pool,       # discard outputs (f16)
            tc.tile_pool(name="sc", bufs=2) as sc_pool,         # accums + scalars
        ):
            for b in range(NBLK):
                r0 = b * P

                xt = [
                    xq_pool.tile([P, CW], F32, name=f"xt{b}_{c}", tag="xq")
                    for c in range(NCH)
                ]
                mt = [
                    m8_pool.tile([P, CW], U8, name=f"mt{b}_{c}", tag="m8")
                    for c in range(NCH)
                ]
                b1 = [
                    b1_pool.tile([P, CW], F16, name=f"b1_{b}_{c}", tag="b1")
                    for c in range(NCH)
                ]
                # accA col = c*3 + {0:cnt, 1:r1, 2:r2}; accB col = c*2 + {0:sab, 1:sq}
                accA = sc_pool.tile([P, 3 * NCH], F32, name=f"accA_{b}", tag="accA")
                accB = sc_pool.tile([P, 2 * NCH], F32, name=f"accB_{b}", tag="accB")
                redA = sc_pool.tile([P, 6], F32, name=f"redA_{b}", tag="redA")
                redB = sc_pool.tile([P, 4], F32, name=f"redB_{b}", tag="redB")
                stA = sc_pool.tile([P, 3], F32, name=f"stA_{b}", tag="stA")
                stB = sc_pool.tile([P, 2], F32, name=f"stB_{b}", tag="stB")
                sv = sc_pool.tile([P, 26], F32, name=f"sv_{b}", tag="sv")

                def col(t, i):
                    return t[:, i : i + 1]

                # ------------- stage A: load, mask, first-order sums ---------
                for c in range(NCH):
                    nc.sync.dma_start(xt[c][:], x[r0 : r0 + P, c * CW : (c + 1) * CW])
                    nc.sync.dma_start(mt[c][:], mk[r0 : r0 + P, c * CW : (c + 1) * CW])
                    # mask cast (value discarded) + cnt partial
                    cnt_t = t16_pool.tile([P, CW], F16, name=f"cnt{b}_{c}", tag="t16")
                    if CNT_ENGINE[c] == "act":
                        nc.scalar.activation(
                            cnt_t[:], mt[c][:], Act.Copy,
                            accum_out=col(accA, c * 3 + 0),
                        )
                    else:
                        nc.vector.tensor_scalar(
                            cnt_t[:], mt[c][:], 1.0, 0.0, Alu.mult, Alu.add,
                            accum_out=col(accA, c * 3 + 0),
                        )
                    # T = x*m (in place) + r1 partial
                    nc.vector.scalar_tensor_tensor(
                        xt[c][:], xt[c][:], 1.0, mt[c][:],
                        Alu.bypass, Alu.mult,
                        accum_out=col(accA, c * 3 + 1),
                    )
                    # r2 partial: sum(T^2); output value discarded
                    sq_t = t16_pool.tile([P, CW], F16, name=f"sq{b}_{c}", tag="t16")
                    if SQ_ENGINE[c] == "act":
                        nc.scalar.activation(
                            sq_t[:], xt[c][:], Act.Square,
                            accum_out=col(accA, c * 3 + 2),
                        )
                    elif SQ_ENGINE[c] == "dve":
                        nc.vector.tensor_tensor_reduce(
                            sq_t[:], xt[c][:], xt[c][:], 1.0, 0.0,
                            Alu.mult, Alu.add,
                            accum_out=col(accA, c * 3 + 2),
                        )
                    else:
                        nc.gpsimd.scalar_tensor_tensor(
                            sq_t[:], xt[c][:], 1.0, xt[c][:],
                            Alu.bypass, Alu.mult,
                            accum_out=col(accA, c * 3 + 2),
                        )

                # reduce accA (chunk-major, 3 quantities) -> stA = [cnt, r1, r2]
                nc.vector.tensor_add(redA[:, 0:6], accA[:, 0:6], accA[:, 6:12])
                nc.vector.tensor_add(stA[:, 0:3], redA[:, 0:3], redA[:, 3:6])
                cnt, r1, r2 = col(stA, 0), col(stA, 1), col(stA, 2)

                cntc, inv, mean1, nm1 = (col(sv, i) for i in range(0, 4))
                e1, v1c, s1, ns1 = (col(sv, i) for i in range(4, 8))
                sg1, am1, ncnt, nqinv = (col(sv, i) for i in range(8, 12))
                sqc, mean2, nm2, nam1 = (col(sv, i) for i in range(12, 16))
                sTm, t1, t2, ns1x2 = (col(sv, i) for i in range(16, 20))
                sq2, e2, v2c, s2 = (col(sv, i) for i in range(20, 24))
                kk = col(sv, 24)

                nc.vector.tensor_scalar(cntc, cnt, 1.0, None, Alu.max)
                nc.vector.reciprocal(inv, cntc)
                nc.vector.tensor_mul(mean1, r1, inv)
                nc.vector.tensor_scalar(nm1, mean1, -1.0, None, Alu.mult)
                nc.vector.tensor_mul(e1, r2, inv)
                # v1c = max((e1 - mean1^2) * C2, EPS)  [two steps]
                nc.vector.scalar_tensor_tensor(
                    v1c, mean1, nm1, e1, Alu.mult, Alu.add
                )
                nc.vector.tensor_scalar(v1c, v1c, C2, EPS, Alu.mult, Alu.max)
                nc.scalar.activation(s1, v1c, Act.Sqrt)
                nc.vector.tensor_scalar(ns1, s1, -1.0, None, Alu.mult)
                # correction scalars
                nc.scalar.activation(sg1, mean1, Act.Sign)
                nc.vector.tensor_mul(am1, mean1, sg1)
                nc.vector.tensor_scalar(ncnt, cnt, -1.0, float(N), Alu.mult, Alu.add)
                nc.vector.scalar_tensor_tensor(
                    nqinv, sg1, ns1, mean1, Alu.mult, Alu.add
                )
                nc.vector.tensor_scalar(nam1, am1, -1.0, None, Alu.mult)

                # ------------- stage B: residual q + accumulators ------------
                for c in range(NCH):
                    ab = ab_pool.tile([P, CW], F32, name=f"ab{b}_{c}", tag="ab")
                    nc.scalar.activation(
                        ab[:], xt[c][:], Act.Abs, bias=nm1,
                        accum_out=col(accB, c * 2 + 0),
                    )
                    nc.scalar.activation(b1[c][:], xt[c][:], Act.Sign, bias=nm1)
                    # q = (ab - s1) * b1, in place onto the T tile
                    nc.vector.scalar_tensor_tensor(
                        xt[c][:], ab[:], ns1, b1[c][:],
                        Alu.add, Alu.mult,
                        accum_out=col(accB, c * 2 + 1),
                    )

                # reduce accB -> stB = [sab, sq]
                nc.vector.tensor_add(redB[:, 0:4], accB[:, 0:4], accB[:, 4:8])
                nc.vector.tensor_add(stB[:, 0:2], redB[:, 0:2], redB[:, 2:4])
                sab, sq = col(stB, 0), col(stB, 1)

                # corrected sums and second-order stats
                nc.vector.scalar_tensor_tensor(sqc, ncnt, nqinv, sq, Alu.mult, Alu.add)
                nc.vector.tensor_mul(mean2, sqc, inv)
                nc.vector.tensor_scalar(nm2, mean2, -1.0, None, Alu.mult)
                nc.vector.scalar_tensor_tensor(sTm, ncnt, nam1, sab, Alu.mult, Alu.add)
                nc.vector.scalar_tensor_tensor(t1, r1, nm1, r2, Alu.mult, Alu.add)
                nc.vector.scalar_tensor_tensor(t2, cnt, v1c, t1, Alu.mult, Alu.add)
                nc.vector.tensor_scalar(ns1x2, s1, -2.0, None, Alu.mult)
                nc.vector.scalar_tensor_tensor(sq2, sTm, ns1x2, t2, Alu.mult, Alu.add)
                nc.vector.tensor_mul(e2, sq2, inv)
                nc.vector.scalar_tensor_tensor(v2c, mean2, nm2, e2, Alu.mult, Alu.add)
                nc.vector.tensor_scalar(v2c, v2c, C2, EPS, Alu.mult, Alu.max)
                nc.scalar.activation(s2, v2c, Act.Sqrt)
                nc.vector.tensor_add(kk, mean1, mean2)

                # ------------- stage C: output assembly ----------------------
                for c in range(NCH):
                    b2 = b2_pool.tile([P, CW], F16, name=f"b2_{b}_{c}", tag="b2")
                    nc.scalar.activation(b2[:], xt[c][:], Act.Sign, bias=nm2)
                    # u = s2*b2 + K (in place; DVE TS runs 4x, Pool offloads)
                    eng2 = nc.gpsimd if U_ENGINE[c] == "pool" else nc.vector
                    eng2.tensor_scalar(b2[:], b2[:], s2, kk, Alu.mult, Alu.add)
                    # bs1 = s1*b1 (TS 4x, in place)
                    nc.vector.tensor_scalar(b1[c][:], b1[c][:], s1, None, Alu.mult)
                    # w = u + bs1 (TT 2x, in place)
                    nc.vector.tensor_add(b2[:], b2[:], b1[c][:])
                    # out = w * m -> f32, overwrites the q tile
                    eng3 = nc.gpsimd if OUT_ENGINE[c] == "pool" else nc.vector
                    eng3.tensor_mul(xt[c][:], b2[:], mt[c][:])
                    nc.sync.dma_start(
                        out[r0 : r0 + P, c * CW : (c + 1) * CW], xt[c][:]
                    )

    return nc


def get_program():
    if "nc" not in _CACHE:
        nc = _build_program()
        nc.finalize()
        _CACHE["nc"] = nc
    return _CACHE["nc"]


def kernel(x: np.ndarray, mask: np.ndarray) -> np.ndarray:
    import time

    from concourse.bass_utils import run_bass_kernel_spmd

    x = np.ascontiguousarray(np.asarray(x, dtype=np.float32))
    mask = np.ascontiguousarray(np.asarray(mask))
    if mask.dtype == np.bool_ or mask.dtype == np.uint8:
        mask_u8 = mask.view(np.uint8)
    else:
        mask_u8 = (mask != 0).astype(np.uint8)
    assert x.shape == (R * NCORES, N), x.shape
    assert mask_u8.shape == (R * NCORES, N), mask_u8.shape

    nc = get_program()
    in_maps = [
        {
            "x": x[k * R : (k + 1) * R],
            "mask": mask_u8[k * R : (k + 1) * R],
        }
        for k in range(NCORES)
    ]
    last_err = None
    for attempt in range(3):
        try:
            res = run_bass_kernel_spmd(nc, in_maps, core_ids=list(range(NCORES)))
            return np.concatenate([r["out"] for r in res.results], axis=0)
        except Exception as e:  # transient NRT/device hiccups
            last_err = e
            if attempt < 2:
                time.sleep(10)
    raise last_err


if __name__ == "__main__":
    xs = np.random.randn(R * NCORES, N).astype(np.float32)
    ms = (np.random.randint(0, 2, (R * NCORES, N))).astype(bool)
    y = kernel(xs, ms)
    print(y.shape, y.dtype)
